# revision 33
# baseline (speedup 1.0000x reference)
"""Trainium2 Bass kernel for nn_Desc_Seq2Seq (2-layer LSTM encoder-decoder).

Self-contained: builds the Bass program, shards the batch 8-ways across
NeuronCores (data-parallel), runs via run_bass_kernel_spmd, gathers output.

Pipeline-optimized v2:
  - Gate order (f, i, g, o): the c-chain (fc -> c -> tanh_c) overlaps the
    i/g/o matmul blocks instead of serializing after them.
  - Per-tick PE order: [x-mms] [per gate j: HH1 k0-3, IH1 k0-3, HH0 k0-3]
    [4 transposes].  x-mms at tick start cover the previous tick's o-tail.
  - o-gate add/act/hnew/transpose/copy split into 4x128-col pieces so the
    first hT piece lands ~1.3us after the last matmul.
  - hnew/transposes in fp16 (single PE pass vs fp32's double pass).
  - Decoder tick: [HH1 x16][IH1 x16][HH0 j01][L1-cell][fc][HH0 j23][inp]
    [L0-cell] so the fc/select chain hides under HH0 matmuls.
"""
from contextlib import ExitStack
import numpy as np
import concourse.bass as bass
import concourse.bacc as bacc
import concourse.tile as tile
from concourse import mybir
from concourse.bass import DynSlice

F32 = mybir.dt.float32
F32R = mybir.dt.float32r
F16 = mybir.dt.float16
AF = mybir.ActivationFunctionType
OP = mybir.AluOpType

H = 512
G = 2048
NCH = 4
KCH = 4
BL = 64
FEAT = 128


def build_kernel(T_enc, pred_len, fc_b_val, enc_unroll=0, dec_unroll=0):
    nc = bacc.Bacc("TRN2", target_bir_lowering=False, debug=False,
                   num_devices=8)

    xT_d = nc.dram_tensor("xT", [T_enc, FEAT, BL], F16, kind="ExternalInput")
    wenc_d = nc.dram_tensor("wenc", [FEAT, 13 * G], F16, kind="ExternalInput")
    wdec_d = nc.dram_tensor("wdec", [FEAT, 12 * G], F16, kind="ExternalInput")
    br_enc1_d = nc.dram_tensor("br_enc1", [2, G], F16, kind="ExternalInput")
    br_enc0_d = nc.dram_tensor("br_enc0", [2, G], F16, kind="ExternalInput")
    br_dec1_d = nc.dram_tensor("br_dec1", [2, G], F16, kind="ExternalInput")
    br_dec0_d = nc.dram_tensor("br_dec0", [2, G], F16, kind="ExternalInput")
    ident_d = nc.dram_tensor("ident", [128, 128], F32, kind="ExternalInput")
    identf_d = nc.dram_tensor("identf", [128, 128], F16, kind="ExternalInput")
    wdi0_d = nc.dram_tensor("wdi0", [1, G], F16, kind="ExternalInput")
    fcw_d = nc.dram_tensor("fcw", [FEAT, KCH], F16, kind="ExternalInput")
    ytf_d = nc.dram_tensor("ytf", [BL, pred_len], F32, kind="ExternalInput")
    tf1m_d = nc.dram_tensor("tf1m", [BL, pred_len], F32, kind="ExternalInput")
    xdec_d = nc.dram_tensor("xdec", [BL, 96 * 8], F32, kind="ExternalInput")
    out_d = nc.dram_tensor("out", [BL, pred_len], F32, kind="ExternalOutput")

    with ExitStack() as ctx:
        tc = ctx.enter_context(tile.TileContext(nc))
        state = ctx.enter_context(tc.tile_pool(name="state", bufs=1))
        psh = ctx.enter_context(tc.tile_pool(name="psh", bufs=2, space="PSUM"))
        ew = ctx.enter_context(tc.tile_pool(name="ew", bufs=2))
        xp = ctx.enter_context(tc.tile_pool(name="xp", bufs=4))

        hT = state.tile([128, KCH * 128], F16)
        c_all = state.tile([128, H], F32)
        ident = state.tile([128, 128], F32)
        identf = state.tile([128, 128], F16)
        br_enc1 = state.tile([2, G], F16)
        br_enc0 = state.tile([2, G], F16)
        br_dec1 = state.tile([2, G], F16)
        br_dec0 = state.tile([2, G], F16)
        ones = state.tile([2, BL], F16)
        wdi0 = state.tile([1, G], F16)
        fcw = state.tile([FEAT, KCH], F16)
        ytf = state.tile([BL, pred_len], F32)
        tf1m = state.tile([BL, pred_len], F32)
        outs = state.tile([BL, pred_len], F32)
        inpT = state.tile([1, BL], F16)

        nc.any.memset(hT[:], 0.0)
        nc.any.memset(c_all[:], 0.0)
        nc.any.memset(ones[:], 1.0)
        nc.sync.dma_start(ident[:], ident_d.ap())
        nc.sync.dma_start(identf[:], identf_d.ap())
        nc.sync.dma_start(br_enc1[:], br_enc1_d.ap())
        nc.sync.dma_start(br_enc0[:], br_enc0_d.ap())
        nc.sync.dma_start(br_dec1[:], br_dec1_d.ap())
        nc.sync.dma_start(br_dec0[:], br_dec0_d.ap())
        nc.sync.dma_start(wdi0[:], wdi0_d.ap())
        nc.sync.dma_start(fcw[:], fcw_d.ap())
        nc.sync.dma_start(ytf[:], ytf_d.ap())
        nc.sync.dma_start(tf1m[:], tf1m_d.ap())

        # decoder weights preloaded during the encoder (SBUF has room)
        wdec_pool = ctx.enter_context(tc.tile_pool(name="wdec", bufs=1))
        wd = wdec_pool.tile([FEAT, 12 * G], F16)
        nc.sync.dma_start(wd[:], wdec_d.ap())

        xT_ap = xT_d.ap()

        def wsl(wt, chunk, j):
            return wt[:, chunk * G + j * H: chunk * G + j * H + H]

        def h1T(k):
            return hT[:, 128 * k: 128 * k + BL]

        def h0T(k):
            return hT[:, 128 * k + BL: 128 * k + 128]

        def load_x(t_iv):
            xr = xp.tile([FEAT, BL], F16, tag="xr")
            nc.sync.dma_start(xr[:], xT_ap[DynSlice(t_iv, 1), :, :].squeeze(0))
            return xr

        def mm1(pg, j, st, w, start, stop):
            nc.tensor.matmul(pg[j][0:BL, :], st, w, start=start, stop=stop,
                             tile_position=(0, 0), skip_group_check=True)

        def mm0(pg, j, st, w, start, stop):
            nc.tensor.matmul(pg[j][BL:128, :], st, w, start=start, stop=stop,
                             tile_position=(0, BL), skip_group_check=True)

        def bias_mm(pg, j, brow, lo, start=True, stop=False):
            """Inject the gate bias through the matmul accumulator:
            ones[1,64] (stationary) x bias_row[1,512] (moving).  These are
            hT-independent, so they double as tick-start PE cover."""
            dst = pg[j][lo:lo + BL, :]
            nc.tensor.matmul(dst, ones[:], brow[:, j * H:(j + 1) * H],
                             start=start, stop=stop, tile_position=(0, lo),
                             skip_group_check=True)

        def emit_cell(pg, lo, hi, copy_dve=False):
            """LSTM cell elementwise for partitions [lo:hi).
            Gate chunks: j0=f(Sig) j1=i(Sig) j2=g(Tanh) j3=o(Sig).
            Bias is already in PSUM (bias_mm), so activations read the
            psum gates directly -- no DVE adds at all.
            o-gate is piece-split (4x128 cols); hnew/transposes fp16.
            fcp runs on GpSimd (Pool, SBUF-only) to unload the DVE."""
            act = ew.tile([128, G], F32, tag="act", name="act")
            fcp = ew.tile([128, H], F32, tag="fcp", name="fcp")
            ig = ew.tile([128, H], F32, tag="ig", name="ig")
            tch = ew.tile([128, H], F32, tag="tch", name="tch")
            hnew = ew.tile([128, H], F16, tag="hnew", name="hnew")
            sl = slice(lo, hi)

            def gact(j, func):
                nc.scalar.activation(act[sl, j * H:(j + 1) * H],
                                     pg[j][sl, :], func)

            gact(0, AF.Sigmoid)          # f
            nc.gpsimd.tensor_tensor(fcp[sl, :], act[sl, 0:H], c_all[sl, :],
                                    OP.mult)
            gact(1, AF.Sigmoid)          # i
            gact(2, AF.Tanh)             # g
            nc.vector.tensor_tensor(ig[sl, :], act[sl, H:2 * H],
                                    act[sl, 2 * H:3 * H], OP.mult)
            for a in range(2):           # c + tanh(c) in 2x256 pieces
                cs = slice(a * 256, (a + 1) * 256)
                nc.vector.tensor_tensor(c_all[sl, cs], fcp[sl, cs],
                                        ig[sl, cs], OP.add)
                nc.scalar.activation(tch[sl, cs], c_all[sl, cs], AF.Tanh)
            # o-gate pieces: act (psum-direct) / hnew / transpose / copy
            phs = []
            for p in range(KCH):
                ps = slice(3 * H + p * 128, 3 * H + p * 128 + 128)
                hs = slice(p * 128, (p + 1) * 128)
                nc.scalar.activation(act[sl, ps], pg[3][sl, hs], AF.Sigmoid)
                nc.vector.tensor_tensor(hnew[sl, hs], act[sl, ps],
                                        tch[sl, hs], OP.mult)
                ph = psh.tile([128, 128], F16, tag="ph", name="ph")
                nc.tensor.transpose(ph[:, sl], hnew[sl, hs], identf[sl, sl])
                phs.append(ph)
                if p >= 1:               # lag copies one piece for pipelining
                    _copy_piece(phs[p - 1], p - 1, lo, hi, copy_dve)
            _copy_piece(phs[3], 3, lo, hi, copy_dve)

        def _copy_piece(ph, p, lo, hi, copy_dve):
            sl = slice(lo, hi)
            if copy_dve:
                nc.vector.tensor_copy(hT[:, 128 * p + lo: 128 * p + hi],
                                      ph[:, sl])
            else:
                nc.scalar.copy(hT[:, 128 * p + lo: 128 * p + hi], ph[:, sl])

        # ---------- encoder ----------
        with tc.tile_pool(name="psg", bufs=6, space="PSUM") as psg, \
             tc.tile_pool(name="wenc", bufs=1) as wenc_pool:
            def alloc_pg():
                return [psg.tile([128, H], F32, tag="pg", name=f"pg{j}",
                                 bufs=6) for j in range(NCH)]

            we = wenc_pool.tile([FEAT, 13 * G], F16)
            nc.sync.dma_start(we[:], wenc_d.ap())
            E_IH0, E_HH0, E_IH1, E_HH1 = 0, 1, 5, 9

            # prologue: L0 step 0 (h0=0, c0=0 -> only the x projection)
            pg0 = alloc_pg()
            xr0 = load_x(0)
            for j in range(NCH):
                bias_mm(pg0, j, br_enc0, BL, start=True)
                mm0(pg0, j, xr0[:], wsl(we, E_IH0, j), False, True)
            emit_cell(pg0, BL, 128, copy_dve=True)

            def enc_tick(iv):
                pg = alloc_pg()
                xr = load_x(iv + 1)
                # bias + x mms first: hT-independent, they open every psum
                # group AND cover the previous tick's o-chain tail.  The
                # (0,0)/(0,64) bias pairs stream concurrently.
                for j in range(NCH):
                    bias_mm(pg, j, br_enc1, 0, start=True)
                    bias_mm(pg, j, br_enc0, BL, start=True)
                for j in range(NCH):
                    mm0(pg, j, xr[:], wsl(we, E_IH0, j), False, False)
                # Two PE lanes: A = tile_position (0,0) (L1: HH1+IH1),
                # B = (0,64) (L0: HH0).  Different column groups run
                # CONCURRENTLY, so interleaving hides B under A.
                A, B = [], []
                for j in range(NCH):
                    for k in range(KCH):
                        A.append(lambda j=j, k=k: mm1(
                            pg, j, h1T(k), wsl(we, E_HH1 + k, j), False,
                            False))
                    for k in range(KCH):
                        A.append(lambda j=j, k=k: mm1(
                            pg, j, h0T(k), wsl(we, E_IH1 + k, j), False,
                            k == KCH - 1))
                for j in range(NCH):
                    for k in range(KCH):
                        B.append(lambda j=j, k=k: mm0(
                            pg, j, h0T(k), wsl(we, E_HH0 + k, j), False,
                            k == KCH - 1))
                for i in range(16):
                    A[2 * i]()
                    A[2 * i + 1]()
                    B[i]()
                emit_cell(pg, 0, 128, copy_dve=True)

            if enc_unroll and T_enc > enc_unroll + 1:
                tc.For_i_unrolled(0, T_enc - 1, 1, enc_tick,
                                  max_unroll=enc_unroll)
            else:
                for t in range(T_enc - 1):
                    enc_tick(t)

            # epilogue: L1 step T-1
            pgE = alloc_pg()
            for j in range(NCH):
                bias_mm(pgE, j, br_enc1, 0, start=True)
                for k in range(KCH):
                    mm1(pgE, j, h1T(k), wsl(we, E_HH1 + k, j), False, False)
                for k in range(KCH):
                    mm1(pgE, j, h0T(k), wsl(we, E_IH1 + k, j), False,
                        k == KCH - 1)
            emit_cell(pgE, 0, BL, copy_dve=True)

        # ---------- decoder ----------
        # Rotating psum pool (no cross-iteration WAR); per-tick PE order:
        #   [HH1 16] [IH1 16] [HH0 j01] [L1-cell T] [fc 4] [HH0 j23]
        #   [sel-T] [inp 4] [L0-cell T]
        # fc/sel/L1-tail each covered by the HH0 blocks behind them.
        D_HH0, D_IH1, D_HH1 = 0, 4, 8

        xdec = state.tile([BL, 96 * 8], F32)
        nc.sync.dma_start(xdec[:], xdec_d.ap())
        dsum = ew.tile([BL, 1], F32, tag="dsum")
        nc.vector.tensor_reduce(dsum[:], xdec[:], axis=mybir.AxisListType.X,
                                op=OP.add)
        pin = psh.tile([128, 128], F32, tag="ph", name="pin")
        nc.tensor.transpose(pin[0:1, 0:BL], dsum[:], ident[0:BL, 0:BL])
        nc.vector.tensor_copy(inpT[:], pin[0:1, 0:BL])

        def fc_mms_and_sel(t, last=False):
            """fc matmuls + outs write (+ select).  Returns sel tile."""
            tsl = DynSlice(t, 1)
            po = psh.tile([128, 128], F32, tag="ph", name="po")
            for k in range(KCH):
                nc.tensor.matmul(po[0:BL, 0:1], h1T(k), fcw[:, k:k + 1],
                                 start=(k == 0), stop=(k == KCH - 1),
                                 tile_position=(0, 0), skip_group_check=True)
            nc.vector.tensor_scalar_add(outs[:, tsl], po[0:BL, 0:1],
                                        fc_b_val)
            if last:
                return None
            # inp_next = (1-tf_t)*out + tf_t*y[t+1]   (ytf = tf*y_next)
            sel = ew.tile([BL, 1], F32, tag="sel")
            nc.vector.scalar_tensor_tensor(sel[:], outs[:, tsl],
                                           tf1m[:, tsl], ytf[:, tsl],
                                           op0=OP.mult, op1=OP.add)
            return sel

        def sel_to_inpT(sel):
            psel = psh.tile([128, 128], F32, tag="ph", name="psel")
            nc.tensor.transpose(psel[0:1, 0:BL], sel[:], ident[0:BL, 0:BL])
            nc.vector.tensor_copy(inpT[:], psel[0:1, 0:BL])

        with tc.tile_pool(name="psd", bufs=6, space="PSUM") as psd:
            def alloc_pgd():
                return [psd.tile([128, H], F32, tag="pg", name=f"pg{j}",
                                 bufs=6) for j in range(NCH)]

            # prologue: L0 step 0 (enc-final h0, dsum input)
            pgD = alloc_pgd()
            for j in range(NCH):
                bias_mm(pgD, j, br_dec0, BL, start=True)
                for k in range(KCH):
                    mm0(pgD, j, h0T(k), wsl(wd, D_HH0 + k, j), False,
                        False)
            for j in range(NCH):
                mm0(pgD, j, inpT[:], wdi0[:, j * H:(j + 1) * H], False,
                    True)
            emit_cell(pgD, BL, 128, copy_dve=True)

            def dec_tick(iv):
                pg = alloc_pgd()
                # bias mms first (hT-independent cover, open all groups)
                for j in range(NCH):
                    bias_mm(pg, j, br_dec1, 0, start=True)
                    bias_mm(pg, j, br_dec0, BL, start=True)
                # A lane (0,0): HH1 + IH1; B lane (0,64): HH0 j0/j1 runs
                # concurrently with the tail of the A lane.
                A, B = [], []
                for j in range(NCH):
                    for k in range(KCH):
                        A.append(lambda j=j, k=k: mm1(
                            pg, j, h1T(k), wsl(wd, D_HH1 + k, j), False,
                            False))
                for j in range(NCH):
                    for k in range(KCH):
                        A.append(lambda j=j, k=k: mm1(
                            pg, j, h0T(k), wsl(wd, D_IH1 + k, j), False,
                            k == KCH - 1))
                for j in (0, 1, 2):
                    for k in range(KCH):
                        B.append(lambda j=j, k=k: mm0(
                            pg, j, h0T(k), wsl(wd, D_HH0 + k, j), False,
                            False))
                for a in A[:8]:
                    a()
                for i in range(12):
                    A[8 + 2 * i]()
                    A[9 + 2 * i]()
                    B[i]()
                emit_cell(pg, 0, BL, copy_dve=True)               # L1
                sel = fc_mms_and_sel(iv)
                for k in range(KCH):
                    mm0(pg, 3, h0T(k), wsl(wd, D_HH0 + k, 3), False,
                        False)
                sel_to_inpT(sel)        # transpose covered by HH0 j23
                for j in range(NCH):
                    mm0(pg, j, inpT[:], wdi0[:, j * H:(j + 1) * H], False,
                        True)
                emit_cell(pg, BL, 128, copy_dve=True)             # L0

            if dec_unroll and pred_len > dec_unroll + 1:
                tc.For_i_unrolled(0, pred_len - 1, 1, dec_tick,
                                  max_unroll=dec_unroll)
            else:
                for t in range(pred_len - 1):
                    dec_tick(t)

            # epilogue: L1 step pred_len-1, final fc
            pgF = alloc_pgd()
            for j in range(NCH):
                bias_mm(pgF, j, br_dec1, 0, start=True)
                for k in range(KCH):
                    mm1(pgF, j, h1T(k), wsl(wd, D_HH1 + k, j), False,
                        False)
                for k in range(KCH):
                    mm1(pgF, j, h0T(k), wsl(wd, D_IH1 + k, j), False,
                        k == KCH - 1)
            emit_cell(pgF, 0, BL)
            fc_mms_and_sel(pred_len - 1, last=True)

        nc.sync.dma_start(out_d.ap(), outs[:])

    nc.compile()
    return nc


# ---------------- host-side packing ----------------
# packed gate order: (f, i, g, o); PyTorch rows are (i, f, g, o)
GATE_ORDER = np.concatenate([np.arange(H, 2 * H),
                             np.arange(0, H),
                             np.arange(2 * H, 3 * H),
                             np.arange(3 * H, 4 * H)])


def pack_w(W):
    return np.ascontiguousarray(np.asarray(W)[GATE_ORDER, :].T
                                .astype(np.float16))


def pack_wenc(Wih0, Whh0, Wih1, Whh1):
    chunks = [pack_w(Wih0)]
    for Wt in (Whh0, Wih1, Whh1):
        t = pack_w(Wt)
        chunks += [np.ascontiguousarray(t[k * 128:(k + 1) * 128])
                   for k in range(4)]
    return np.ascontiguousarray(np.concatenate(chunks, axis=1))


def pack_wdec(Whh0, Wih1, Whh1):
    chunks = []
    for Wt in (Whh0, Wih1, Whh1):
        t = pack_w(Wt)
        chunks += [np.ascontiguousarray(t[k * 128:(k + 1) * 128])
                   for k in range(4)]
    return np.ascontiguousarray(np.concatenate(chunks, axis=1))


def pack_bias_row(b):
    bo = np.asarray(b, np.float64)[GATE_ORDER]
    hi = bo.astype(np.float16)
    res = (bo - hi.astype(np.float64)).astype(np.float16)
    return np.ascontiguousarray(np.stack([hi, res]))


def make_in_map(core, T_enc, pred_len, inp):
    sl = slice(core * BL, core * BL + BL)
    tf = ((np.asarray(inp["tf_mask"])[:pred_len] != 0)
          & (np.arange(pred_len) < pred_len - 1)).astype(np.float32)
    y_next = np.zeros((BL, pred_len), np.float32)
    y_next[:, :pred_len - 1] = np.asarray(inp["y"])[sl, 1:pred_len, 0]
    ytf = np.ascontiguousarray(y_next * tf[None, :])
    tf1m = np.ascontiguousarray(np.broadcast_to(1.0 - tf, (BL, pred_len))
                                .astype(np.float32))
    xT = np.ascontiguousarray(
        np.asarray(inp["X_encode"])[sl, :T_enc].transpose(1, 2, 0)
        .astype(np.float16))
    return {
        "xT": xT,
        "wenc": pack_wenc(inp["enc_W_ih0"], inp["enc_W_hh0"],
                          inp["enc_W_ih1"], inp["enc_W_hh1"]),
        "wdec": pack_wdec(inp["dec_W_hh0"], inp["dec_W_ih1"],
                          inp["dec_W_hh1"]),
        "br_enc1": pack_bias_row(np.asarray(inp["enc_b_ih1"]) + np.asarray(inp["enc_b_hh1"])),
        "br_enc0": pack_bias_row(np.asarray(inp["enc_b_ih0"]) + np.asarray(inp["enc_b_hh0"])),
        "br_dec1": pack_bias_row(np.asarray(inp["dec_b_ih1"]) + np.asarray(inp["dec_b_hh1"])),
        "br_dec0": pack_bias_row(np.asarray(inp["dec_b_ih0"]) + np.asarray(inp["dec_b_hh0"])),
        "ident": np.eye(128, dtype=np.float32),
        "identf": np.eye(128, dtype=np.float16),
        "wdi0": np.ascontiguousarray(
            np.asarray(inp["dec_W_ih0"])[GATE_ORDER, 0][None, :]
            .astype(np.float16)),
        "fcw": np.ascontiguousarray(
            np.asarray(inp["fc_W"])[0].reshape(4, 128).T.astype(np.float16)),
        "ytf": ytf, "tf1m": tf1m,
        "xdec": np.ascontiguousarray(
            np.asarray(inp["X_decode"])[sl].reshape(BL, -1)
            .astype(np.float32)),
    }


def unpack_out(results, pred_len):
    full = np.zeros((8 * BL, pred_len, 1), np.float32)
    for c in range(8):
        full[c * BL:(c + 1) * BL, :, 0] = results[c]["out"]
    return full


# ---------------- public entry point ----------------
_NC_CACHE = {}


def _get_nc(T_enc, pred_len, fc_b_val):
    key = (T_enc, pred_len, float(fc_b_val))
    if key not in _NC_CACHE:
        _NC_CACHE[key] = build_kernel(T_enc, pred_len, float(fc_b_val),
                                      enc_unroll=8, dec_unroll=16)
    return _NC_CACHE[key]


def kernel(**inputs):
    from concourse.bass_utils import run_bass_kernel_spmd
    inp = {k: np.asarray(v) for k, v in inputs.items()}
    B, T_enc, _ = inp["X_encode"].shape
    pred_len = inp["y"].shape[1]
    assert B == 8 * BL, f"expected batch {8*BL}, got {B}"
    nc = _get_nc(T_enc, pred_len, float(inp["fc_b"][0]))
    in_maps = [make_in_map(c, T_enc, pred_len, inp) for c in range(8)]
    res = run_bass_kernel_spmd(nc, in_maps, core_ids=list(range(8)))
    return unpack_out(res.results, pred_len).astype(np.float32)
